# revision 19
# baseline (speedup 1.0000x reference)
"""GatedDIP forward on 8 Trainium2 NeuronCores (Bass/Tile) — v2.

Sharding: pure data parallel, B=16 -> 2 images/core.
Engine split per core:
  PE: separable Gaussian blur (bf16 matmuls) + pass-2 accumulation of all
      per-channel terms into PSUM via diag-coefficient matmuls.
  DVE: bf16 map algebra (4x-mode tensor_scalar), strided-subsample min/max
      stats (branch norms tolerate ~1e-4; tolerance is 2e-2).
  ScalarE: sin/recip/ln/exp activations (tables batched: trig -> reciprocal ->
      natural_log_exp, 3 loads), PSUM copy-outs.
  Pool: kth_largest top-k threshold, broadcasts, bf16 cast-DMAs, collectives.
Two tiny AllReduces (branch stats, final out stats) implement the reference's
global-batch norm01 semantics.
"""
import contextlib
import math
import sys

import numpy as np

for _p in ("/opt/trn_rl_repo", "/opt/trn_rl_repo/concourse"):
    if _p not in sys.path:
        sys.path.insert(0, _p)

import concourse.mybir as mybir
from concourse import bacc, bass_isa
from concourse.bass_utils import run_bass_kernel_spmd
from concourse.tile import TileContext

F32 = mybir.dt.float32
BF16 = mybir.dt.bfloat16
OP = mybir.AluOpType
AF = mybir.ActivationFunctionType
AX = mybir.AxisListType

B, C, H, W = 16, 3, 512, 512
NCORES = 8
BPC = B // NCORES
HW = H * W
NP_ = 128
FD = HW // NP_            # 2048
FD3 = 3 * FD              # 6144
KSIZE, SIGMA = 13, 2.55
PAD = KSIZE // 2
NUMPX = HW // 1000        # 262
CS = 8
NH = 40
NEG_INF = -3.0e38
POS_INF = 3.0e38
SS = 4                    # stat subsample stride

_OMQ = (2 * (NUMPX - 2) + 1) * (2**31) // (HW - 1) + 1
KTH_Q = 1.0 - _OMQ / 4294967296.0

TONE_CI = [i / 8.0 for i in range(1, 8)]


def _build_T():
    half = (KSIZE - 1) * 0.5
    xs = np.linspace(-half, half, KSIZE)
    k = np.exp(-0.5 * (xs / SIGMA) ** 2)
    k = (k / k.sum()).astype(np.float32)
    T = np.zeros((H, H), dtype=np.float32)
    for m in range(H):
        for t in range(KSIZE):
            r = m + t - PAD
            if r < 0:
                r = -r
            elif r > H - 1:
                r = 2 * (H - 1) - r
            T[r, m] += k[t]
    return T


def _tr(x, lo, hi):
    return (np.tanh(x) * 0.5 + 0.5) * (hi - lo) + lo


def _host_consts(latent, w):
    """[B, NH] per-image host scalars."""
    lat = np.asarray(latent, np.float32)
    gate = _tr(lat @ np.asarray(w["gate_w"]).T + np.asarray(w["gate_b"]), 0.01, 1.0)
    wb = np.exp(_tr(lat @ np.asarray(w["wb_w"]).T + np.asarray(w["wb_b"]), -0.5, 0.5))
    cs = 1.0 / (1e-05 + 0.27 * wb[:, 0] + 0.67 * wb[:, 1] + 0.06 * wb[:, 2])
    wb = cs[:, None] * wb
    lg = math.log(2.5)
    gamma = np.exp(_tr(lat @ np.asarray(w["gamma_w"]).T + np.asarray(w["gamma_b"]), -lg, lg))[:, 0]
    y = _tr(lat @ np.asarray(w["sharp_w"]).T + np.asarray(w["sharp_b"]), 0.1, 1.0)[:, 0]
    om = _tr(lat @ np.asarray(w["defog_w"]).T + np.asarray(w["defog_b"]), 0.1, 1.0)[:, 0]
    al = np.tanh(lat @ np.asarray(w["contrast_w"]).T + np.asarray(w["contrast_b"]))[:, 0]
    tc = _tr((lat @ np.asarray(w["tone_w"]).T + np.asarray(w["tone_b"])).reshape(-1, CS), 0.5, 2.0)
    tsc = CS / (tc.sum(axis=1) + 1e-30)
    hc = np.zeros((B, NH), dtype=np.float32)
    hc[:, 0] = gate[:, 0]
    hc[:, 1] = gate[:, 1]
    hc[:, 2] = gate[:, 3]
    hc[:, 3] = gate[:, 4]
    hc[:, 4] = gate[:, 5]
    hc[:, 5] = gate[:, 6]
    hc[:, 6] = gate[:, 2]
    hc[:, 7:10] = wb
    hc[:, 10] = gamma
    hc[:, 11] = 1.0 + y
    hc[:, 12] = y / (1.0 + y)
    hc[:, 13] = -om
    hc[:, 14] = 0.5 * al
    hc[:, 15] = 1.0 - al
    hc[:, 16] = tsc * tc[:, 0]
    d = np.diff(tc, axis=1)
    for i in range(1, 8):
        hc[:, 16 + i] = tsc * d[:, i - 1]
    hc[:, 24] = 1.0 - om
    return hc


# hc column indices
HG2, HWB, HGAM, HY1, HYR, HNOM, HALH, HOMAL, HTC0, HD, HOM1 = \
    6, 7, 10, 11, 12, 13, 14, 15, 16, 17, 24

# coef per-image block layout (stride 52)
CK, CB, CMSB, CA4, CA5, CLNA1, CE = 0, 3, 6, 7, 8, 9, 10  # CE..CE+6 tone e_t
CGAM = 17

# stats tile [128, 32]: cols 0..15 mins (negated later), 16..31 maxs
SX0, SJ0, SSH, SCT = 0, 6, 12, 14
SMX = 16


def _emit(tc, nc, xs, hcs, T16s, eyes, outs, no_collective=False, dbgs=None):
    ctx = contextlib.ExitStack()
    with ctx:
        persist = ctx.enter_context(tc.tile_pool(name="persist", bufs=1))
        xpool = ctx.enter_context(tc.tile_pool(name="xp", bufs=1))
        scrp = ctx.enter_context(tc.tile_pool(name="scr", bufs=7))
        scr32p = ctx.enter_context(tc.tile_pool(name="scr32", bufs=1))
        auxp = ctx.enter_context(tc.tile_pool(name="aux", bufs=5))
        psump = ctx.enter_context(tc.tile_pool(name="psum", bufs=2, space="PSUM"))
        psbig = ctx.enter_context(tc.tile_pool(name="psbig", bufs=2, space="PSUM"))
        psmall = ctx.enter_context(tc.tile_pool(name="psmall", bufs=1, space="PSUM"))
        dram = ctx.enter_context(tc.tile_pool(name="dram", bufs=1, space="DRAM"))

        V = nc.vector
        S = nc.scalar
        G = nc.gpsimd
        PE = nc.tensor

        _scrn = [0]

        def scr16():
            _scrn[0] += 1
            return scrp.tile([NP_, FD], BF16, tag="s16", name=f"s{_scrn[0]}")

        def scr32():
            _scrn[0] += 1
            return scr32p.tile([NP_, FD], F32, tag="s32", name=f"t{_scrn[0]}")

        # ---------- persistent tiles ----------
        # X32 slot is reused by OUT after phase 1a (same tag).
        X32 = [xpool.tile([NP_, FD3], F32, tag=f"X{i}", name=f"X32_{i}")
               for i in range(BPC)]
        X16 = [persist.tile([NP_, FD3], BF16, tag=f"Xs{i}", name=f"X16_{i}")
               for i in range(BPC)]
        SB = [persist.tile([NP_, FD3], BF16, tag=f"SB{i}", name=f"SB{i}")
              for i in range(BPC)]
        Jt = [persist.tile([NP_, FD3], BF16, tag=f"J{i}", name=f"J{i}")
              for i in range(BPC)]
        Rr = [persist.tile([NP_, FD], BF16, tag=f"R{i}", name=f"R{i}") for i in range(BPC)]
        Wm = [persist.tile([NP_, FD], BF16, tag=f"W{i}", name=f"W{i}") for i in range(BPC)]
        LM, SN, RL, TP = [[None] * BPC for _ in range(4)]

        def aux(name):
            return auxp.tile([NP_, FD], BF16, tag="aux", name=name)
        T16 = persist.tile([NP_, 4 * H], BF16, tag="T16", name="T16")
        EYE = persist.tile([NP_, NP_], BF16, tag="eye", name="eye")
        ones = persist.tile([NP_, 1], F32, tag="ones", name="ones")
        hcrow = persist.tile([1, 2 * NH], F32, tag="hcrow", name="hcrow")
        bch = persist.tile([NP_, 2 * NH], F32, tag="bch", name="bch")
        acc3 = [persist.tile([NP_, 3], F32, tag=f"acc3{i}", name=f"acc3{i}") for i in range(BPC)]
        arow = [persist.tile([1, 3], F32, tag=f"arow{i}", name=f"arow{i}") for i in range(BPC)]
        bca = persist.tile([NP_, 12], F32, tag="bca", name="bca")
        kout = [persist.tile([NP_, 2], F32, tag=f"kout{i}", name=f"kout{i}") for i in range(BPC)]
        vbc = [persist.tile([NP_, 1], F32, tag=f"vbc{i}", name=f"vbc{i}") for i in range(BPC)]
        stats = persist.tile([NP_, 32], F32, tag="stats", name="stats")
        stats_r = persist.tile([NP_, 32], F32, tag="stats_r", name="stats_r")
        coll = persist.tile([NP_, 12], F32, tag="coll", name="coll")
        gst = persist.tile([NP_, 12], F32, tag="gst", name="gst")
        gstrow = persist.tile([1, 12], F32, tag="gstrow", name="gstrow")
        coef = persist.tile([NP_, 104], F32, tag="coef", name="coef")
        tmp = persist.tile([NP_, 16], F32, tag="tmp", name="tmp")
        stats2 = persist.tile([NP_, 8], F32, tag="stats2", name="stats2")
        stats2_r = persist.tile([NP_, 8], F32, tag="stats2_r", name="stats2_r")
        coll2 = persist.tile([NP_, 2], F32, tag="coll2", name="coll2")
        gst2row = persist.tile([1, 2], F32, tag="gst2row", name="gst2row")
        gst2 = persist.tile([NP_, 2], F32, tag="gst2", name="gst2")
        osob = persist.tile([NP_, 2], F32, tag="osob", name="osob")
        cbias = persist.tile([NP_, 1], F32, tag="cbias", name="cbias")
        negone = persist.tile([NP_, 1], F32, tag="negone", name="negone")
        nbca = persist.tile([NP_, 6], F32, tag="nbca", name="nbca")
        ND = 13  # diag tiles per image: [K0 K1 K2 msb a4 a5 e0..e6]
        DG = [persist.tile([NP_, ND * NP_], BF16, tag=f"DG{i}", name=f"DG{i}")
              for i in range(BPC)]

        V.memset(ones[:], 1.0)
        V.memset(cbias[:], math.pi / 2)
        V.memset(negone[:], -1.0)

        # ---------- loads ----------
        for i in range(BPC):
            nc.sync.dma_start(out=hcrow[0:1, i * NH:(i + 1) * NH], in_=hcs[i:i + 1, :])
        G.partition_broadcast(bch[:], hcrow[0:1, :])
        nc.sync.dma_start(out=T16[:], in_=T16s.rearrange("(b p) m -> p b m", p=NP_))
        nc.sync.dma_start(out=EYE[:], in_=eyes)
        for i in range(BPC):
            for c in range(C):
                nc.sync.dma_start(
                    out=X32[i][:, c * FD:(c + 1) * FD],
                    in_=xs[i, c].rearrange("(b p) w -> p b w", p=NP_))
                G.dma_start(
                    out=X16[i][:, c * FD:(c + 1) * FD],
                    in_=xs[i, c].rearrange("(b p) w -> p b w", p=NP_))

        def hcc(i, col_):
            return bch[:, i * NH + col_:i * NH + col_ + 1]

        def x32c(i, c):
            return X32[i][:, c * FD:(c + 1) * FD]

        def x16c(i, c):
            return X16[i][:, c * FD:(c + 1) * FD]

        def sbc(i, c):
            return SB[i][:, c * FD:(c + 1) * FD]

        def jc(i, c):
            return Jt[i][:, c * FD:(c + 1) * FD]

        def col(t, j, n=1):
            return t[:, j:j + n]

        def sub4(ap):
            return ap.rearrange("p (a b) -> p a b", b=SS)[:, :, 0:1]

        def st_min(dst, ap):
            V.tensor_reduce(out=dst, in_=sub4(ap), axis=AX.XY, op=OP.min)

        def st_max(dst, ap):
            V.tensor_reduce(out=dst, in_=sub4(ap), axis=AX.XY, op=OP.max)

        def full_stat16(dst, ap, op):
            t = scr16()
            V.tensor_tensor(out=t[:, 0:1024], in0=ap[:, 0:1024],
                            in1=ap[:, 1024:2048], op=op)
            V.tensor_tensor(out=t[:, 1024:1536], in0=t[:, 0:512],
                            in1=t[:, 512:1024], op=op)
            V.tensor_reduce(out=dst, in_=t[:, 1024:1536], axis=AX.X, op=op)

        def full_stat32(dst, ap, op):
            t = scr32()
            V.tensor_tensor(out=t[:, 0:1024], in0=ap[:, 0:1024],
                            in1=ap[:, 1024:2048], op=op)
            V.tensor_tensor(out=t[:, 1024:1536], in0=t[:, 0:512],
                            in1=t[:, 512:1024], op=op)
            V.tensor_reduce(out=dst, in_=t[:, 1024:1536], axis=AX.X, op=op)

        def fold(dst, ap, op):
            st = col(tmp, 15)
            V.tensor_reduce(out=st, in_=sub4(ap), axis=AX.XY,
                            op=OP.min if op == "min" else OP.max)
            V.tensor_tensor(out=dst, in0=dst, in1=st,
                            op=OP.min if op == "min" else OP.max)

        # ================= PHASE 1 per image =================
        for i in range(BPC):
            # dark channel (fp32, exact for top-k) + threshold + masked sums
            dark = scr32()
            V.tensor_tensor(out=dark[:], in0=x32c(i, 0), in1=x32c(i, 1), op=OP.min)
            V.tensor_tensor(out=dark[:], in0=dark[:], in1=x32c(i, 2), op=OP.min)
            G.kth_largest(kout[i][:], dark[:], n_per_lane=FD, k=NUMPX, quantile=KTH_Q)
            G.partition_broadcast(vbc[i][:], kout[i][0:1, 1:2])
            for c in range(C):
                mscr = scr16()
                V.scalar_tensor_tensor(
                    out=mscr[:], in0=dark[:], scalar=vbc[i][:, 0:1], in1=x32c(i, c),
                    op0=OP.is_gt, op1=OP.mult, accum_out=col(acc3[i], c))
            ps = psmall.tile([NP_, 8], F32, tag="ps", name="ps")
            PE.matmul(out=ps[0:1, 0:3], lhsT=ones[:], rhs=acc3[i][:, 0:3],
                      start=True, stop=True, skip_group_check=True)
            V.tensor_scalar(out=arow[i][:], in0=ps[0:1, 0:3], scalar1=1.0 / NUMPX,
                            scalar2=None, op0=OP.mult)
            G.partition_broadcast(bca[:, i * 6:i * 6 + 3], arow[i][:])
            V.reciprocal(out=bca[:, i * 6 + 3:i * 6 + 6], in_=bca[:, i * 6:i * 6 + 3])

            # x16 per-channel min/max (strided)
            for c in range(C):
                st_min(col(stats, SX0 + 3 * i + c), x16c(i, c))
                st_max(col(stats, SMX + SX0 + 3 * i + c), x16c(i, c))

            # blur on PE (bf16): H-pass -> Z, W-pass -> SB scaled y/(1+y)
            for c in range(C):
                Z = scr16()
                for q in range(4):
                    pz = psump.tile([NP_, H], F32, tag="pz", name="pz")
                    for b in range(4):
                        lhsT = X16[i][:, c * FD + b * W + q * NP_:
                                      c * FD + b * W + (q + 1) * NP_]
                        lo = max(0, 128 * b - PAD)
                        hi = min(H, 128 * b + 128 + PAD)
                        ov = 128 * b + PAD
                        if b == 0:
                            PE.matmul(out=pz[:, lo:hi], lhsT=lhsT,
                                      rhs=T16[:, b * H + lo:b * H + hi],
                                      start=True, stop=(b == 3), skip_group_check=True)
                        else:
                            PE.matmul(out=pz[:, lo:ov], lhsT=lhsT,
                                      rhs=T16[:, b * H + lo:b * H + ov],
                                      start=False, stop=False, skip_group_check=True)
                            PE.matmul(out=pz[:, ov:hi], lhsT=lhsT,
                                      rhs=T16[:, b * H + ov:b * H + hi],
                                      start=True, stop=(b == 3), skip_group_check=True)
                    S.copy(out=Z[:, q * H:(q + 1) * H], in_=pz[:])
                for s in range(4):
                    pf = psump.tile([NP_, W], F32, tag="pz", name="pf")
                    for q in range(4):
                        lhsT = Z[:, q * H + s * NP_:q * H + (s + 1) * NP_]
                        lo = max(0, 128 * q - PAD)
                        hi = min(W, 128 * q + 128 + PAD)
                        ov = 128 * q + PAD
                        if q == 0:
                            PE.matmul(out=pf[:, lo:hi], lhsT=lhsT,
                                      rhs=T16[:, q * H + lo:q * H + hi],
                                      start=True, stop=(q == 3), skip_group_check=True)
                        else:
                            PE.matmul(out=pf[:, lo:ov], lhsT=lhsT,
                                      rhs=T16[:, q * H + lo:q * H + ov],
                                      start=False, stop=False, skip_group_check=True)
                            PE.matmul(out=pf[:, ov:hi], lhsT=lhsT,
                                      rhs=T16[:, q * H + ov:q * H + hi],
                                      start=True, stop=(q == 3), skip_group_check=True)
                    S.activation(out=SB[i][:, c * FD + s * W:c * FD + (s + 1) * W],
                                 in_=pf[:], func=AF.Copy, bias=0.0, scale=hcc(i, HYR))

            # sharp stats: v' = x16 - SB per channel (true v = (1+y) v')
            for c in range(C):
                vt = scr16()
                V.tensor_tensor(out=vt[:], in0=x16c(i, c), in1=sbc(i, c), op=OP.subtract)
                if c == 0:
                    st_min(col(stats, SSH + i), vt[:])
                    st_max(col(stats, SMX + SSH + i), vt[:])
                else:
                    fold(col(stats, SSH + i), vt[:], "min")
                    fold(col(stats, SMX + SSH + i), vt[:], "max")

            # fog A: p_c = x/a - 1 ; t'' = max((1-om) - om*min_c p_c, 0.01)
            m0 = scr16()
            S.activation(out=m0[:], in_=x16c(i, 0), func=AF.Identity,
                         bias=negone[:, 0:1], scale=bca[:, i * 6 + 3:i * 6 + 4])
            m1 = scr16()
            S.activation(out=m1[:], in_=x16c(i, 1), func=AF.Identity,
                         bias=negone[:, 0:1], scale=bca[:, i * 6 + 4:i * 6 + 5])
            V.tensor_tensor(out=m0[:], in0=m0[:], in1=m1[:], op=OP.min)
            S.activation(out=m1[:], in_=x16c(i, 2), func=AF.Identity,
                         bias=negone[:, 0:1], scale=bca[:, i * 6 + 5:i * 6 + 6])
            V.tensor_tensor(out=m0[:], in0=m0[:], in1=m1[:], op=OP.min)
            V.tensor_scalar(out=m1[:], in0=m0[:], scalar1=hcc(i, HNOM),
                            scalar2=hcc(i, HOM1), op0=OP.mult, op1=OP.add)
            TP[i] = aux(f"TP{i}")
            V.tensor_scalar(out=TP[i][:], in0=m1[:], scalar1=0.01, scalar2=None,
                            op0=OP.max)
            with nc.allow_low_precision(reason="bf16 maps; tolerance 2e-2"):
                V.reciprocal(out=Rr[i][:], in_=TP[i][:])

            # contrast A: lum chain -> lumc = min(lum,1)+1e-6
            t0 = scr16()
            S.activation(out=t0[:], in_=x16c(i, 2), func=AF.Copy, bias=0.0,
                         scale=0.06)
            t1 = scr16()
            S.activation(out=t1[:], in_=x16c(i, 1), func=AF.Copy, bias=0.0,
                         scale=0.67)
            V.tensor_tensor(out=t0[:], in0=t0[:], in1=t1[:], op=OP.add)
            t1b = scr16()
            S.activation(out=t1b[:], in_=x16c(i, 0), func=AF.Copy, bias=0.0,
                         scale=0.27)
            V.tensor_tensor(out=t0[:], in0=t0[:], in1=t1b[:], op=OP.add)
            LM[i] = aux(f"LM{i}")
            V.tensor_scalar(out=LM[i][:], in0=t0[:], scalar1=1.0, scalar2=1e-6,
                            op0=OP.min, op1=OP.add)

        # ===== batched ScalarE trig/recip =====
        for i in range(BPC):
            SN[i] = aux(f"SN{i}")
            S.activation(out=SN[i][:], in_=LM[i][:], func=AF.Sin,
                         bias=cbias[:, 0:1], scale=-math.pi)
        with nc.allow_low_precision(reason="bf16 maps; tolerance 2e-2"):
            for i in range(BPC):
                RL[i] = aux(f"RL{i}")
                V.reciprocal(out=RL[i][:], in_=LM[i][:])

        # ===== finish contrast W + ct/j maps and stats =====
        for i in range(BPC):
            # w1n = 0.5al*sn - 0.5al ; W = -w1n*rl + (1-al)
            w1 = scr16()
            V.tensor_scalar(out=w1[:], in0=SN[i][:], scalar1=hcc(i, HALH),
                            scalar2=hcc(i, HALH), op0=OP.mult, op1=OP.subtract)
            w2 = scr16()
            V.tensor_tensor(out=w2[:], in0=w1[:], in1=RL[i][:], op=OP.mult)
            V.tensor_scalar(out=Wm[i][:], in0=w2[:], scalar1=-1.0,
                            scalar2=hcc(i, HOMAL), op0=OP.mult, op1=OP.add)
            for c in range(C):
                ctm = scr16()
                V.tensor_tensor(out=ctm[:], in0=x16c(i, c), in1=Wm[i][:], op=OP.mult)
                if i == 0 and c == 0:
                    full_stat16(col(stats, SCT + 0), ctm[:], OP.min)
                    full_stat16(col(stats, SMX + SCT + 0), ctm[:], OP.max)
                    V.tensor_copy(out=col(stats, SCT + 1), in_=col(stats, SCT))
                    V.tensor_copy(out=col(stats, SMX + SCT + 1),
                                  in_=col(stats, SMX + SCT))
                else:
                    full_stat16(col(tmp, 14), ctm[:], OP.min)
                    V.tensor_tensor(out=col(stats, SCT + 0), in0=col(stats, SCT + 0),
                                    in1=col(tmp, 14), op=OP.min)
                    full_stat16(col(tmp, 14), ctm[:], OP.max)
                    V.tensor_tensor(out=col(stats, SMX + SCT + 0),
                                    in0=col(stats, SMX + SCT + 0),
                                    in1=col(tmp, 14), op=OP.max)
            # j maps (kept for pass 2) + per-channel strided stats
            V.tensor_scalar(out=nbca[:, i * 3:i * 3 + 3],
                            in0=bca[:, i * 6:i * 6 + 3], scalar1=-1.0,
                            scalar2=None, op0=OP.mult)
            for c in range(C):
                xst = scr16()
                S.activation(out=xst[:], in_=x16c(i, c), func=AF.Identity,
                             bias=nbca[:, i * 3 + c:i * 3 + c + 1], scale=1.0)
                V.tensor_tensor(out=jc(i, c), in0=xst[:], in1=Rr[i][:], op=OP.mult)
                full_stat16(col(stats, SJ0 + 3 * i + c), jc(i, c), OP.min)
                full_stat16(col(stats, SMX + SJ0 + 3 * i + c), jc(i, c), OP.max)

        # ================= STATS -> COLLECTIVE 1 =================
        V.tensor_scalar(out=stats[:, 0:16], in0=stats[:, 0:16], scalar1=-1.0,
                        scalar2=None, op0=OP.mult)
        G.partition_all_reduce(out_ap=stats_r[:], in_ap=stats[:], channels=NP_,
                               reduce_op=bass_isa.ReduceOp.max)

        # derived stats -> coll [negmins: wb gm sh fg ct tn | maxs same]
        for i in range(BPC):
            nxmn = stats_r[:, SX0 + 3 * i:SX0 + 3 * i + 3]
            xmx = stats_r[:, SMX + SX0 + 3 * i:SMX + SX0 + 3 * i + 3]
            njmn = stats_r[:, SJ0 + 3 * i:SJ0 + 3 * i + 3]
            jmx = stats_r[:, SMX + SJ0 + 3 * i:SMX + SJ0 + 3 * i + 3]
            a3c = bca[:, i * 6:i * 6 + 3]
            V.tensor_tensor(out=tmp[:, 0:3], in0=bch[:, i * NH + HWB:i * NH + HWB + 3],
                            in1=nxmn, op=OP.mult)
            V.tensor_reduce(out=col(tmp, 3), in_=tmp[:, 0:3], axis=AX.X, op=OP.max)
            V.tensor_tensor(out=tmp[:, 0:3], in0=bch[:, i * NH + HWB:i * NH + HWB + 3],
                            in1=xmx, op=OP.mult)
            V.tensor_reduce(out=col(tmp, 4), in_=tmp[:, 0:3], axis=AX.X, op=OP.max)
            V.tensor_reduce(out=col(tmp, 5), in_=nxmn, axis=AX.X, op=OP.max)
            V.tensor_reduce(out=col(tmp, 6), in_=xmx, axis=AX.X, op=OP.max)
            V.tensor_scalar(out=col(tmp, 7), in0=col(tmp, 5), scalar1=-1.0,
                            scalar2=None, op0=OP.mult)
            V.tensor_scalar(out=col(tmp, 8), in0=col(tmp, 7), scalar1=1e-4,
                            scalar2=None, op0=OP.max)
            S.activation(out=col(tmp, 8), in_=col(tmp, 8), func=AF.Ln)
            V.tensor_scalar(out=col(tmp, 9), in0=col(tmp, 6), scalar1=1e-4,
                            scalar2=None, op0=OP.max)
            S.activation(out=col(tmp, 9), in_=col(tmp, 9), func=AF.Ln)
            S.activation(out=col(tmp, 8), in_=col(tmp, 8), func=AF.Exp,
                         scale=hcc(i, HGAM))
            S.activation(out=col(tmp, 9), in_=col(tmp, 9), func=AF.Exp,
                         scale=hcc(i, HGAM))
            for (vcl, ocl) in ((7, 10), (6, 11)):
                V.tensor_scalar(out=col(tmp, ocl), in0=col(tmp, vcl),
                                scalar1=hcc(i, HTC0), scalar2=None, op0=OP.mult)
                for t in range(7):
                    V.tensor_scalar(out=col(tmp, 12), in0=col(tmp, vcl),
                                    scalar1=TONE_CI[t], scalar2=0.0,
                                    op0=OP.subtract, op1=OP.max)
                    V.tensor_scalar(out=col(tmp, 13), in0=col(tmp, 12),
                                    scalar1=hcc(i, HD + t), scalar2=None, op0=OP.mult)
                    V.tensor_tensor(out=col(tmp, ocl), in0=col(tmp, ocl),
                                    in1=col(tmp, 13), op=OP.add)
            V.tensor_tensor(out=tmp[:, 0:3], in0=njmn, in1=a3c, op=OP.subtract)
            V.tensor_reduce(out=col(tmp, 13), in_=tmp[:, 0:3], axis=AX.X, op=OP.max)
            V.tensor_tensor(out=tmp[:, 0:3], in0=jmx, in1=a3c, op=OP.add)
            V.tensor_reduce(out=col(tmp, 14), in_=tmp[:, 0:3], axis=AX.X, op=OP.max)
            # sharp scaled by (1+y_i)
            V.tensor_tensor(out=col(tmp, 0), in0=col(stats_r, SSH + i),
                            in1=hcc(i, HY1), op=OP.mult)
            V.tensor_tensor(out=col(tmp, 1), in0=col(stats_r, SMX + SSH + i),
                            in1=hcc(i, HY1), op=OP.mult)
            if i == 0:
                V.tensor_copy(out=col(coll, 0), in_=col(tmp, 3))
                V.tensor_copy(out=col(coll, 6), in_=col(tmp, 4))
                V.tensor_copy(out=col(coll, 1), in_=col(tmp, 8))
                V.tensor_copy(out=col(coll, 7), in_=col(tmp, 9))
                V.tensor_copy(out=col(coll, 5), in_=col(tmp, 10))
                V.tensor_copy(out=col(coll, 11), in_=col(tmp, 11))
                V.tensor_copy(out=col(coll, 3), in_=col(tmp, 13))
                V.tensor_copy(out=col(coll, 9), in_=col(tmp, 14))
                V.tensor_copy(out=col(coll, 2), in_=col(tmp, 0))
                V.tensor_copy(out=col(coll, 8), in_=col(tmp, 1))
            else:
                V.tensor_tensor(out=col(coll, 0), in0=col(coll, 0), in1=col(tmp, 3), op=OP.max)
                V.tensor_tensor(out=col(coll, 6), in0=col(coll, 6), in1=col(tmp, 4), op=OP.max)
                V.tensor_tensor(out=col(coll, 1), in0=col(coll, 1), in1=col(tmp, 8), op=OP.min)
                V.tensor_tensor(out=col(coll, 7), in0=col(coll, 7), in1=col(tmp, 9), op=OP.max)
                V.tensor_tensor(out=col(coll, 5), in0=col(coll, 5), in1=col(tmp, 10), op=OP.min)
                V.tensor_tensor(out=col(coll, 11), in0=col(coll, 11), in1=col(tmp, 11), op=OP.max)
                V.tensor_tensor(out=col(coll, 3), in0=col(coll, 3), in1=col(tmp, 13), op=OP.max)
                V.tensor_tensor(out=col(coll, 9), in0=col(coll, 9), in1=col(tmp, 14), op=OP.max)
                V.tensor_tensor(out=col(coll, 2), in0=col(coll, 2), in1=col(tmp, 0), op=OP.max)
                V.tensor_tensor(out=col(coll, 8), in0=col(coll, 8), in1=col(tmp, 1), op=OP.max)
        V.tensor_scalar(out=col(coll, 1), in0=col(coll, 1), scalar1=-1.0,
                        scalar2=None, op0=OP.mult)
        V.tensor_scalar(out=col(coll, 5), in0=col(coll, 5), scalar1=-1.0,
                        scalar2=None, op0=OP.mult)
        # contrast: global already folded into SCT+0 (negated via stats block)
        V.tensor_copy(out=col(coll, 4), in_=col(stats_r, SCT))
        V.tensor_copy(out=col(coll, 10), in_=col(stats_r, SMX + SCT))

        cin = dram.tile([1, 12], F32, tag="cin", name="cin")
        cout = dram.tile([1, 12], F32, tag="cout", name="cout")
        nc.sync.dma_start(out=cin[:], in_=coll[0:1, :])
        if no_collective:
            nc.sync.dma_start(out=cout[:], in_=cin[:])
        else:
            G.collective_compute("AllReduce", OP.max,
                                 replica_groups=[list(range(NCORES))],
                                 ins=[cin[:].opt()], outs=[cout[:].opt()])
        nc.sync.dma_start(out=gstrow[:], in_=cout[:])
        G.partition_broadcast(gst[:], gstrow[:])

        # gamma ln for image 0 hoisted: runs during collective-1 latency
        lnx0 = []
        for c in range(C):
            cl = scr16()
            V.tensor_scalar(out=cl[:], in0=x16c(0, c), scalar1=1e-4,
                            scalar2=None, op0=OP.max)
            S.activation(out=cl[:], in_=cl[:], func=AF.Ln)
            lnx0.append(cl)

        # ================= COEFFICIENTS + DIAGS =================
        V.tensor_tensor(out=tmp[:, 0:6], in0=gst[:, 6:12], in1=gst[:, 0:6], op=OP.add)
        V.reciprocal(out=tmp[:, 6:12], in_=tmp[:, 0:6])
        for i in range(BPC):
            cb = i * 52
            al6 = coef[:, cb + 40:cb + 46]
            V.tensor_tensor(out=al6, in0=tmp[:, 6:12],
                            in1=bch[:, i * NH:i * NH + 6], op=OP.mult)
            V.tensor_tensor(out=coef[:, cb + 26:cb + 32], in0=al6, in1=gst[:, 0:6],
                            op=OP.mult)
            V.tensor_reduce(out=col(tmp, 12), in_=coef[:, cb + 26:cb + 32],
                            axis=AX.X, op=OP.add)
            V.tensor_tensor(out=col(tmp, 13), in0=al6[:, 2:3], in1=hcc(i, HY1), op=OP.mult)
            V.tensor_tensor(out=col(tmp, 14), in0=al6[:, 5:6], in1=hcc(i, HTC0), op=OP.mult)
            V.tensor_tensor(out=col(tmp, 13), in0=col(tmp, 13), in1=col(tmp, 14), op=OP.add)
            V.tensor_tensor(out=col(tmp, 13), in0=col(tmp, 13), in1=hcc(i, HG2), op=OP.add)
            V.tensor_scalar(out=coef[:, cb + CK:cb + CK + 3],
                            in0=bch[:, i * NH + HWB:i * NH + HWB + 3],
                            scalar1=al6[:, 0:1], scalar2=None, op0=OP.mult)
            V.tensor_scalar(out=coef[:, cb + CK:cb + CK + 3],
                            in0=coef[:, cb + CK:cb + CK + 3],
                            scalar1=col(tmp, 13), scalar2=None, op0=OP.add)
            V.tensor_scalar(out=coef[:, cb + CB:cb + CB + 3],
                            in0=bca[:, i * 6:i * 6 + 3], scalar1=al6[:, 3:4],
                            scalar2=col(tmp, 12), op0=OP.mult, op1=OP.add)
            V.tensor_tensor(out=col(tmp, 13), in0=al6[:, 2:3], in1=hcc(i, HY1),
                            op=OP.mult)
            V.tensor_scalar(out=coef[:, cb + CMSB:cb + CMSB + 1], in0=col(tmp, 13),
                            scalar1=-1.0, scalar2=None, op0=OP.mult)
            V.tensor_copy(out=coef[:, cb + CA4:cb + CA4 + 1], in_=al6[:, 3:4])
            V.tensor_copy(out=coef[:, cb + CA5:cb + CA5 + 1], in_=al6[:, 4:5])
            S.activation(out=coef[:, cb + CLNA1:cb + CLNA1 + 1], in_=al6[:, 1:2],
                         func=AF.Ln)
            V.tensor_scalar(out=coef[:, cb + CE:cb + CE + 7],
                            in0=bch[:, i * NH + HD:i * NH + HD + 7],
                            scalar1=al6[:, 5:6], scalar2=None, op0=OP.mult)
            V.tensor_copy(out=coef[:, cb + CGAM:cb + CGAM + 1], in_=hcc(i, HGAM))
            dcols = [cb + CK, cb + CK + 1, cb + CK + 2, cb + CMSB, cb + CA4,
                     cb + CA5] + [cb + CE + t for t in range(7)]
            for d, cc in enumerate(dcols):
                V.tensor_scalar(out=DG[i][:, d * NP_:(d + 1) * NP_], in0=EYE[:],
                                scalar1=coef[:, cc:cc + 1], scalar2=None,
                                op0=OP.mult)

        def dgt(i, d):
            return DG[i][:, d * NP_:(d + 1) * NP_]

        # ================= PASS 2 =================
        OUT = [xpool.tile([NP_, FD3], F32, tag=f"X{i}", name=f"OUT{i}")
               for i in range(BPC)]
        for i in range(BPC):
            cb = i * 52
            if i == 0:
                lnx = lnx0
            else:
                lnx = []
                for c in range(C):
                    cl = scr16()
                    V.tensor_scalar(out=cl[:], in0=x16c(i, c), scalar1=1e-4,
                                    scalar2=None, op0=OP.max)
                    S.activation(out=cl[:], in_=cl[:], func=AF.Ln)
                    lnx.append(cl)
            for c in range(C):
                et = scr16()
                S.activation(out=et[:], in_=lnx[c][:], func=AF.Exp,
                             scale=coef[:, cb + CGAM:cb + CGAM + 1],
                             bias=coef[:, cb + CLNA1:cb + CLNA1 + 1])
                ctt = scr16()
                V.tensor_tensor(out=ctt[:], in0=x16c(i, c), in1=Wm[i][:], op=OP.mult)
                uts = []
                for t in range(7):
                    ut = scr16()
                    V.tensor_scalar(out=ut[:], in0=x16c(i, c), scalar1=TONE_CI[t],
                                    scalar2=0.0, op0=OP.subtract, op1=OP.max)
                    uts.append(ut)
                oc = OUT[i][:, c * FD:(c + 1) * FD]
                for h in range(2):
                    Ph = psbig.tile([NP_, 1024], F32, tag="P", name="P")
                    lo = h * 1024

                    def acc_h(d, rhs, start, stop=False):
                        for q in range(2):
                            PE.matmul(out=Ph[:, q * 512:(q + 1) * 512],
                                      lhsT=dgt(i, d),
                                      rhs=rhs[:, lo + q * 512:lo + (q + 1) * 512],
                                      start=start, stop=stop,
                                      skip_group_check=True)

                    acc_h(0 + c, x16c(i, c), start=True)
                    acc_h(3, sbc(i, c), start=False)
                    acc_h(4, jc(i, c), start=False)
                    acc_h(5, ctt[:], start=False)
                    for t in range(7):
                        acc_h(6 + t, uts[t][:], start=False)
                    for q in range(2):
                        PE.matmul(out=Ph[:, q * 512:(q + 1) * 512], lhsT=EYE[:],
                                  rhs=et[:, lo + q * 512:lo + (q + 1) * 512],
                                  start=False, stop=True, skip_group_check=True)
                    S.copy(out=oc[:, lo:lo + 1024], in_=Ph[:])
                full_stat32(col(stats2, 3 * i + c), oc, OP.min)
                full_stat32(col(tmp, 0), oc, OP.max)
                V.tensor_scalar(out=col(tmp, 0), in0=col(tmp, 0),
                                scalar1=coef[:, cb + CB + c:cb + CB + c + 1],
                                scalar2=None, op0=OP.add)
                if i == 0 and c == 0:
                    V.tensor_copy(out=col(stats2, 6), in_=col(tmp, 0))
                else:
                    V.tensor_tensor(out=col(stats2, 6), in0=col(stats2, 6),
                                    in1=col(tmp, 0), op=OP.max)

        # ================= COLLECTIVE 2 + PASS 3 =================
        V.tensor_scalar(out=stats2[:, 0:6], in0=stats2[:, 0:6], scalar1=-1.0,
                        scalar2=None, op0=OP.mult)
        for i in range(BPC):
            V.tensor_copy(out=tmp[:, 3 * i:3 * i + 3],
                          in_=coef[:, i * 52 + CB:i * 52 + CB + 3])
        V.tensor_tensor(out=stats2[:, 0:6], in0=stats2[:, 0:6], in1=tmp[:, 0:6],
                        op=OP.subtract)
        V.memset(col(stats2, 7), 0.0)
        G.partition_all_reduce(out_ap=stats2_r[:], in_ap=stats2[:], channels=NP_,
                               reduce_op=bass_isa.ReduceOp.max)
        V.tensor_reduce(out=col(coll2, 0), in_=stats2_r[:, 0:6], axis=AX.X, op=OP.max)
        V.tensor_copy(out=col(coll2, 1), in_=col(stats2_r, 6))
        c2in = dram.tile([1, 2], F32, tag="c2in", name="c2in")
        c2out = dram.tile([1, 2], F32, tag="c2out", name="c2out")
        nc.sync.dma_start(out=c2in[:], in_=coll2[0:1, :])
        if no_collective:
            nc.sync.dma_start(out=c2out[:], in_=c2in[:])
        else:
            G.collective_compute("AllReduce", OP.max,
                                 replica_groups=[list(range(NCORES))],
                                 ins=[c2in[:].opt()], outs=[c2out[:].opt()])
        nc.sync.dma_start(out=gst2row[:], in_=c2out[:])
        G.partition_broadcast(gst2[:], gst2row[:])
        V.tensor_tensor(out=col(osob, 0), in0=gst2[:, 1:2], in1=gst2[:, 0:1], op=OP.add)
        V.reciprocal(out=col(osob, 0), in_=col(osob, 0))
        V.tensor_tensor(out=col(osob, 1), in0=gst2[:, 0:1], in1=col(osob, 0), op=OP.mult)
        V.tensor_scalar(out=tmp[:, 6:12], in0=tmp[:, 0:6], scalar1=col(osob, 0),
                        scalar2=col(osob, 1), op0=OP.mult, op1=OP.add)
        for i in range(BPC):
            for c in range(C):
                oc = OUT[i][:, c * FD:(c + 1) * FD]
                V.tensor_scalar(out=oc, in0=oc, scalar1=col(osob, 0),
                                scalar2=tmp[:, 6 + 3 * i + c:7 + 3 * i + c],
                                op0=OP.mult, op1=OP.add)
                nc.sync.dma_start(
                    out=outs[i, c].rearrange("(b p) w -> p b w", p=NP_),
                    in_=oc)


_PROGRAM_CACHE = {}


def build_program():
    if "nc" in _PROGRAM_CACHE:
        return _PROGRAM_CACHE["nc"]
    nc = bacc.Bacc("TRN2", target_bir_lowering=False, debug=False,
                   num_devices=NCORES)
    x_d = nc.dram_tensor("x", [BPC, C, H, W], F32, kind="ExternalInput")
    hc_d = nc.dram_tensor("hc", [BPC, NH], F32, kind="ExternalInput")
    t16_d = nc.dram_tensor("T16", [H, W], BF16, kind="ExternalInput")
    eye_d = nc.dram_tensor("eye", [NP_, NP_], BF16, kind="ExternalInput")
    out_d = nc.dram_tensor("out", [BPC, C, H, W], F32, kind="ExternalOutput")
    with TileContext(nc) as tc:
        _emit(tc, nc, x_d.ap(), hc_d.ap(), t16_d.ap(), eye_d.ap(), out_d.ap())
    nc.compile()
    _PROGRAM_CACHE["nc"] = nc
    return nc


def make_in_maps(inputs):
    import ml_dtypes
    x = np.ascontiguousarray(np.asarray(inputs["x"], dtype=np.float32))
    w = {k: np.asarray(v, dtype=np.float32) for k, v in inputs.items() if k != "x"}
    hc = _host_consts(w["latent"], w)
    T16 = _build_T().astype(ml_dtypes.bfloat16)
    eye = np.eye(NP_, dtype=np.float32).astype(ml_dtypes.bfloat16)
    return [
        {"x": x[i * BPC:(i + 1) * BPC], "hc": hc[i * BPC:(i + 1) * BPC],
         "T16": T16, "eye": eye}
        for i in range(NCORES)
    ]


def kernel(**inputs):
    nc = build_program()
    in_maps = make_in_maps(inputs)
    res = run_bass_kernel_spmd(nc, in_maps, core_ids=list(range(NCORES)))
    out = np.concatenate([res.results[i]["out"] for i in range(NCORES)], axis=0)
    return out


if __name__ == "__main__":
    import jax
    import reference as R

    with jax.default_device(jax.devices("cpu")[0]):
        inp = R.setup_inputs()
        exp = np.asarray(R.reference(**inp))
    got = kernel(**inp)
    err = np.abs(got - exp).max()
    print("max abs err:", err, "rel:", err / np.abs(exp).max())
